# revision 1
# baseline (speedup 1.0000x reference)
"""CenterLoss kernel for Trainium2 (8 NeuronCores, SPMD data-parallel over B).

Algorithm
---------
reference computes:
    counts[c] = #{i: y_i = c};  sums[c] = sum_{i: y_i = c} f_i
    means = sums / max(counts, 1);  present = counts > 0
    n_c = present ? 0.5*centers_c + 0.5*means_c : centers_c
    loss = 0.5 * mean_i ||f_i - n_{y_i}||^2

Expanding the loss (every class that appears in the batch is present):
    B * 2 * loss = S1 - 0.5*A - 0.75*X + 0.25*W
where
    S1 = sum_i ||f_i||^2
    A  = sum_c sums_c . centers_c
    X  = sum_{c present} ||sums_c||^2 / counts_c
    W  = sum_c counts_c * ||centers_c||^2

So the only heavy device work is the segment sums/counts over feats
(B=131072, D=256, C=1000) and S1.  Each core takes B/8 rows and computes:
  - partial segment sums+counts via one-hot matmuls on the PE
    (one-hot built on DVE from an iota table, feats converted fp32->fp16 on
    ACT; counts ride along as a 257th all-ones column of the rhs)
  - partial S1 via ACT Square activation with free-dim accumulation
The host sums the 8 partial [1024,257] tensors + 8 partial S1 vectors and
evaluates the tiny [C,D] closed form above (the gather/unshard step).
"""

import sys

sys.path.insert(0, "/opt/trn_rl_repo")

import numpy as np

# problem shape (hardcoded per the harness contract)
B, D, C = 131072, 256, 1000
N_CORES = 8
BS = B // N_CORES  # 16384 rows per core
P = 128
G = 4  # row-tiles per DMA group
TILES = BS // P  # 128
GROUPS = TILES // G  # 32
CPAD = 1024  # padded class count
CCHUNKS = CPAD // P  # 8
NFREE = D + 1  # 256 feat cols + 1 ones col for counts
FSTRIDE = 264  # fp16 sub-tile stride (4B aligned, 16B padded)
TAILG = 4  # trailing groups processed chunk-outer (store/compute overlap)

_CACHE: dict = {}


def _build_program():
    import concourse.bacc as bacc
    import concourse.bass as bass
    from concourse import mybir
    from concourse.tile import TileContext

    nc = bacc.Bacc("TRN2", target_bir_lowering=False)

    feats = nc.dram_tensor("feats", [BS, D], mybir.dt.float32, kind="ExternalInput")
    labels_in = nc.dram_tensor(
        "labels", [P, TILES], mybir.dt.float16, kind="ExternalInput"
    )
    # [128 x (8*257 sums+counts | 1 s1)]; stored per chunk so early stores
    # overlap the tail matmuls
    out_sums = nc.dram_tensor(
        "out_sums", [P, CCHUNKS * NFREE + 1], mybir.dt.float32, kind="ExternalOutput"
    )

    feats_ap = feats[:]

    with TileContext(nc) as tc:
        with (
            tc.tile_pool(name="const", bufs=1) as const,
            tc.tile_pool(name="fin", bufs=4) as fin,
            tc.tile_pool(name="sq", bufs=2) as sqp,
            tc.tile_pool(name="f16p", bufs=TAILG + 2) as f16p,
            tc.tile_pool(name="ohp", bufs=4 * TAILG + 6) as ohp,
            tc.tile_pool(name="accp", bufs=1) as accp,
            tc.tile_pool(name="psp", bufs=1, space="PSUM") as psp,
        ):
            # labels DMA (fp16, converted to fp32 on DVE: tensor_scalar
            # is_equal needs an fp32 scalar operand); iota built on the
            # otherwise-idle GPSIMD engine, converted int32 -> fp16 on DVE
            labels16_t = const.tile([P, TILES], mybir.dt.float16, tag="labels16_t")
            nc.sync.dma_start(out=labels16_t[:], in_=labels_in[:])
            labels_t = const.tile([P, TILES], mybir.dt.float32, tag="labels_t")
            nc.vector.tensor_copy(out=labels_t[:], in_=labels16_t[:])
            iota_i = const.tile([P, CPAD], mybir.dt.int32, tag="iota_i")
            nc.gpsimd.iota(iota_i[:], pattern=[[1, CPAD]], channel_multiplier=0)
            iota_f = const.tile([P, CPAD], mybir.dt.float16, tag="iota_f")
            nc.vector.tensor_copy(out=iota_f[:], in_=iota_i[:])
            iota_t = iota_f[:]

            tail_ohs, tail_f16gs = [], []
            # persistent accumulators
            # one column per (group, extra-half): 32 + 3 split extras
            s1cols = accp.tile([P, GROUPS + 3], mybir.dt.float32, tag="s1cols")
            s1_extra_col = [GROUPS]  # next free extra column
            psums = [
                psp.tile(
                    [P, NFREE], mybir.dt.float32, tag=f"psum{k}", name=f"psum{k}"
                )
                for k in range(CCHUNKS)
            ]
            # HAM warm-up: the PE runs at the cold 1.2 GHz clock until ~3.4us
            # of sustained activity. The head leaves PE idle until ~4.6us, so
            # the first ~19 real matmuls would run at half clock. Issue dummy
            # matmuls (zeroed operands, results discarded by the real
            # start=True PSUM clear) from ~0.5us so the real stream is warm.
            warm = const.tile([P, NFREE], mybir.dt.float16, tag="warm")
            nc.vector.memset(warm[:1, :1], 0.0)  # touch so Tile allocates it
            for w in range(12):
                nc.tensor.matmul(
                    out=psums[0][:],
                    lhsT=warm[:, 0:P],
                    rhs=warm[:],
                    start=True,
                    stop=True,
                )

            for t in range(GROUPS):
                # load a [P, G, D] group of feats rows (rows t*512 .. t*512+511).
                # Groups 0/1 are split into smaller loads/conversions so the
                # first matmul starts as soon as the first 128 rows land.
                f16g = f16p.tile([P, G, FSTRIDE], mybir.dt.float16, tag="f16g")
                if t == 0:
                    halves = ((0, 1), (1, 1), (2, 2))
                elif t == 1:
                    halves = ((0, 2), (2, 2))
                else:
                    halves = ((0, G),)
                for h, (off, gh) in enumerate(halves):
                    fg = fin.tile(
                        [P, gh, D], mybir.dt.float32, tag="fg", name="fg"
                    )
                    # very first load rides the ACT HWDGE ring so its
                    # descriptor-gen overlaps the labels DMA's on the SP ring
                    dma_eng = nc.scalar if t == 0 else nc.sync
                    dma_eng.dma_start(
                        out=fg[:],
                        in_=bass.AP(
                            tensor=feats_ap.tensor,
                            offset=(t * G + off) * P * D,
                            ap=[[D, P], [P * D, gh], [1, D]],
                        ),
                    )
                    # fp32 -> fp16 conversion (ACT)
                    nc.scalar.copy(
                        out=f16g[:, off : off + gh, 0:D], in_=fg[:]
                    )
                    # S1 partial: sum over free dim of feats^2 (ACT square+accum)
                    sqt = sqp.tile([P, gh, D], mybir.dt.float32, tag="sqt", name="sqt")
                    if h == 0:
                        col = t
                    else:
                        col = s1_extra_col[0]
                        s1_extra_col[0] += 1
                    nc.scalar.activation(
                        out=sqt[:],
                        in_=fg[:],
                        func=mybir.ActivationFunctionType.Square,
                        accum_out=s1cols[:, col : col + 1],
                    )
                # ones column for counts (DVE)
                nc.vector.memset(f16g[:, :, D : D + 1], 1.0)

                ohs = []
                for s in range(G):
                    j = t * G + s
                    oh = ohp.tile([P, CPAD], mybir.dt.float16, tag="oh")
                    nc.vector.tensor_scalar(
                        oh[:],
                        iota_t,
                        labels_t[:, j : j + 1],
                        None,
                        mybir.AluOpType.is_equal,
                    )
                    ohs.append(oh)
                if t < GROUPS - TAILG:
                    for s in range(G):
                        rhs = f16g[:, s, 0:NFREE]
                        for k in range(CCHUNKS):
                            nc.tensor.matmul(
                                out=psums[k][:],
                                lhsT=ohs[s][:, k * P : (k + 1) * P],
                                rhs=rhs,
                                start=(t == 0 and s == 0),
                                stop=False,
                            )
                else:
                    tail_ohs.append(ohs)
                    tail_f16gs.append(f16g)
            # last TAILG groups: chunk-outer order so chunk k's accumulation
            # closes early and its evacuation/store overlaps the remaining
            # chunks' matmuls
            for k in range(CCHUNKS):
                for g, (ohs_g, f16g_g) in enumerate(zip(tail_ohs, tail_f16gs)):
                    for s in range(G):
                        nc.tensor.matmul(
                            out=psums[k][:],
                            lhsT=ohs_g[s][:, k * P : (k + 1) * P],
                            rhs=f16g_g[:, s, 0:NFREE],
                            start=False,
                            stop=(g == TAILG - 1 and s == G - 1),
                        )

            # write back partials (PSUM -> SBUF -> DRAM; DMA can't read PSUM)
            ev = accp.tile([P, CCHUNKS * NFREE + 1], mybir.dt.float32, tag="ev")
            nc.vector.tensor_reduce(
                out=ev[:, CCHUNKS * NFREE : CCHUNKS * NFREE + 1],
                in_=s1cols[:],
                axis=mybir.AxisListType.X,
                op=mybir.AluOpType.add,
            )
            for k in range(CCHUNKS):
                dst = ev[:, k * NFREE : (k + 1) * NFREE]
                if k % 2 == 0:
                    nc.vector.tensor_copy(out=dst, in_=psums[k][:])
                else:
                    nc.scalar.copy(out=dst, in_=psums[k][:])
            # per-chunk stores: chunks close ~1.7 us apart (chunk-outer tail),
            # so early stores hide under compute and the last piece is small
            for k in range(CCHUNKS):
                lo = k * NFREE
                hi = (k + 1) * NFREE + (1 if k == CCHUNKS - 1 else 0)
                nc.sync.dma_start(out=out_sums[:, lo:hi], in_=ev[:, lo:hi])

    nc.compile()
    return nc


def _get_program():
    if "nc" not in _CACHE:
        _CACHE["nc"] = _build_program()
    return _CACHE["nc"]


def _run_device(feats_np: np.ndarray, labels_np: np.ndarray, trace: bool = False):
    """Shard over cores, run the SPMD bass kernel, return per-core results."""
    from concourse.bass_utils import run_bass_kernel_spmd

    nc = _get_program()
    in_maps = []
    for c in range(N_CORES):
        fshard = np.ascontiguousarray(feats_np[c * BS : (c + 1) * BS])
        lshard = labels_np[c * BS : (c + 1) * BS]
        # [P, TILES]; fp16 is exact for labels < 2048
        ltile = np.ascontiguousarray(lshard.reshape(TILES, P).T.astype(np.float16))
        in_maps.append({"feats": fshard, "labels": ltile})
    kw = {}
    if trace:
        kw = {"trace": True}
    try:
        return run_bass_kernel_spmd(nc, in_maps, core_ids=list(range(N_CORES)), **kw)
    except Exception:
        # transient axon/terminal faults have been observed; retry once
        import time

        time.sleep(2.0)
        return run_bass_kernel_spmd(nc, in_maps, core_ids=list(range(N_CORES)), **kw)


def kernel(feats, centers, labels, _trace: bool = False, _return_res: bool = False):
    feats = np.asarray(feats, dtype=np.float32)
    centers = np.asarray(centers, dtype=np.float32)
    labels_i = np.asarray(labels).astype(np.int64)

    res = _run_device(feats, labels_i, trace=_trace)

    # host combine (the gather/unshard step): tiny [C, D] math
    sums_all = np.zeros((CPAD, NFREE), dtype=np.float64)
    S1 = 0.0
    for c in range(N_CORES):
        raw = res.results[c]["out_sums"]
        part = (
            raw[:, : CCHUNKS * NFREE]
            .reshape(P, CCHUNKS, NFREE)
            .transpose(1, 0, 2)
            .reshape(CPAD, NFREE)
        )
        sums_all += part.astype(np.float64)
        S1 += float(raw[:, CCHUNKS * NFREE].sum())
    sums = sums_all[:C, :D]
    counts = sums_all[:C, D]

    c64 = centers.astype(np.float64)
    A = float((sums * c64).sum())
    present = counts > 0
    X = float((np.square(sums).sum(axis=1)[present] / counts[present]).sum())
    W = float((counts * np.square(c64).sum(axis=1)).sum())
    loss = 0.5 / B * (S1 - 0.5 * A - 0.75 * X + 0.25 * W)
    out = np.float32(loss)
    if _return_res:
        return out, res
    return out



# revision 16
# speedup vs baseline: 2.7115x; 2.7115x over previous
"""CenterLoss kernel for Trainium2 (8 NeuronCores, label-range sharding).

Algorithm
---------
reference computes:
    counts[c] = #{i: y_i = c};  sums[c] = sum_{i: y_i = c} f_i
    means = sums / max(counts, 1);  present = counts > 0
    n_c = present ? 0.5*centers_c + 0.5*means_c : centers_c
    loss = 0.5 * mean_i ||f_i - n_{y_i}||^2

Expanding the loss (every class that appears in the batch is present):
    B * 2 * loss = S1 - 0.5*A - 0.75*X + 0.25*W
where
    S1 = sum_i ||f_i||^2
    A  = sum_c sums_c . centers_c
    X  = sum_{c present} ||sums_c||^2 / counts_c
    W  = sum_c counts_c * ||centers_c||^2

Device work: segment sums over feats (B=131072, D=256, C=1000) and S1.

Sharding: rows are sharded by LABEL RANGE (class-aligned cuts balancing row
counts, ~125 classes / ~16384 rows per core).  Every row in a core then has a
label inside one 128-wide class window, so the one-hot segment-sum matmul
needs a single [128rows x 128cls] stationary per 128-row tile accumulating
into ONE [128, 256] PSUM tile -- 8x less PE work than padding the one-hot to
1024 classes, and no inter-core reduction (classes are disjoint; the host
just concatenates the per-core sums).  counts come from a host bincount.

feats are staged to HBM as fp16 (exact enough: final rel err ~1e-6 vs the
fp32 reference; the check budget is 2e-2), which halves the HBM traffic; the
kernel is then DMA-bound at ~8.7 MB / 360 GB/s ~= 24 us per core.  S1
squares are split ACT/DVE and one-hot builds split DVE/Pool so every
compute engine stays under the DMA window.
"""

import sys

sys.path.insert(0, "/opt/trn_rl_repo")

import numpy as np

# problem shape (hardcoded per the harness contract)
B, D, C = 131072, 256, 1000
N_CORES = 8
P = 128
TG = 4  # row-tiles per DMA group
BS_PAD = 16896  # padded rows per core (16384 + 512 slack for shard imbalance)
TILES = BS_PAD // P  # 132
GROUPS = TILES // TG  # 33
NFREE = D + 1  # 256 sums cols + 1 S1 col in the output
PAD_LABEL = 127.0  # relative label for padded rows (feats are 0 -> no-op)

# engine split knobs (tuned against the TimelineSim trace)
SQ_ON_DVE = frozenset(range(0, GROUPS, 3))  # 11 groups' squares on DVE
OH_POOL_PER_GROUP = 1  # one-hots per group built on Pool (rest on DVE)
N_DVE = len(SQ_ON_DVE)
N_ACT = GROUPS - N_DVE
NSTAT = 12 * N_DVE  # bn_stats words shipped per partition (2 x [P,512] calls)
NOUT = NFREE + NSTAT

_CACHE: dict = {}


def _build_program():
    import concourse.bacc as bacc
    import concourse.bass as bass
    from concourse import mybir
    from concourse.tile import TileContext

    nc = bacc.Bacc("TRN2", target_bir_lowering=False)

    # feats pre-tiled on host: [P, TILES*D], row p holds tile-rows
    # (t*128+p for all t) concatenated -> group loads are 1 descriptor
    # per partition (TG*D*2 = 2 KB contiguous)
    feats = nc.dram_tensor(
        "feats", [P, TILES * D], mybir.dt.float16, kind="ExternalInput"
    )
    labels_in = nc.dram_tensor(
        "labels", [P, TILES], mybir.dt.float16, kind="ExternalInput"
    )
    # [128 x (256 local-class sums | 1 s1 partial | bn_stats words)]
    out_sums = nc.dram_tensor(
        "out_sums", [P, NOUT], mybir.dt.float32, kind="ExternalOutput"
    )

    feats_ap = feats[:]

    with TileContext(nc) as tc:
        with (
            tc.tile_pool(name="const", bufs=1) as const,
            tc.tile_pool(name="fin", bufs=4) as fin,
            tc.tile_pool(name="sq", bufs=2) as sqp,
            tc.tile_pool(name="ohp", bufs=12) as ohp,
            tc.tile_pool(name="accp", bufs=1) as accp,
            tc.tile_pool(name="psp", bufs=1, space="PSUM") as psp,
        ):
            # labels DMA (fp16; converted to fp32 on DVE because tensor_scalar
            # is_equal needs an fp32 scalar operand); iota on the Pool engine
            labels16_t = const.tile([P, TILES], mybir.dt.float16, tag="labels16_t")
            nc.scalar.dma_start(out=labels16_t[:], in_=labels_in[:])
            labels_t = const.tile([P, TILES], mybir.dt.float32, tag="labels_t")
            nc.vector.tensor_copy(out=labels_t[:], in_=labels16_t[:])
            iota_i = const.tile([P, P], mybir.dt.int32, tag="iota_i")
            nc.gpsimd.iota(iota_i[:], pattern=[[1, P]], channel_multiplier=0)
            iota_f = const.tile([P, P], mybir.dt.float16, tag="iota_f")
            nc.vector.tensor_copy(out=iota_f[:], in_=iota_i[:])
            iota_t = iota_f[:]

            # persistent accumulators (s1cols: one column per ACT group;
            # DVE groups ship raw bn_stats instead)
            s1cols = accp.tile([P, N_ACT], mybir.dt.float32, tag="s1cols")
            stats = accp.tile([P, 2 * N_DVE, 6], mybir.dt.float32, tag="stats")
            # bn_stats record layout can vary with AP lowering; zero-fill so
            # unwritten slots contribute 0 to the host-side sum(x^2)
            nc.vector.memset(stats[:], 0.0)
            psum = psp.tile([P, D], mybir.dt.float32, tag="psum", name="psum")

            # HAM warm-up: the PE p-state ramps with sustained activity; issue
            # dummy matmuls early so the real stream runs at full clock.
            # Results land in psum but are discarded by the first start=True.
            warm = const.tile([P, D], mybir.dt.float16, tag="warm")
            nc.vector.memset(warm[:], 0.0)
            for w in range(12):
                nc.tensor.matmul(
                    out=psum[:],
                    lhsT=warm[:, 0:P],
                    rhs=warm[:],
                    start=True,
                    stop=True,
                )

            act_col = 0
            dve_idx = 0
            for g in range(GROUPS):
                # load a [P, TG*D] group of tile-rows (1 descriptor/partition)
                fg = fin.tile([P, TG * D], mybir.dt.float16, tag="fg", name="fg")
                nc.sync.dma_start(
                    out=fg[:],
                    in_=bass.AP(
                        tensor=feats_ap.tensor,
                        offset=g * TG * D,
                        ap=[[TILES * D, P], [1, TG * D]],
                    ),
                )
                # S1 partial for the group: ACT groups do Square+accum; DVE
                # groups emit bn_stats (count/mean/count*var per 256-col
                # sub-tile), from which the host recovers sum(x^2)
                if g in SQ_ON_DVE:
                    for h in range(2):
                        nc.vector.bn_stats(
                            out=stats[:, dve_idx * 2 + h],
                            in_=fg[:, h * 2 * D : (h + 1) * 2 * D],
                        )
                    dve_idx += 1
                else:
                    sqt = sqp.tile([P, TG * D], mybir.dt.float16, tag="sqt", name="sqt")
                    nc.scalar.activation(
                        out=sqt[:],
                        in_=fg[:],
                        func=mybir.ActivationFunctionType.Square,
                        accum_out=s1cols[:, act_col : act_col + 1],
                    )
                    act_col += 1
                # one-hots (DVE at 4x; a share on Pool) + segment matmuls
                for s in range(TG):
                    j = g * TG + s
                    oh = ohp.tile([P, P], mybir.dt.float16, tag="oh")
                    eng = nc.gpsimd if s < OH_POOL_PER_GROUP else nc.vector
                    eng.tensor_scalar(
                        oh[:],
                        iota_t,
                        labels_t[:, j : j + 1],
                        None,
                        mybir.AluOpType.is_equal,
                    )
                    nc.tensor.matmul(
                        out=psum[:],
                        lhsT=oh[:],
                        rhs=fg[:, s * D : (s + 1) * D],
                        start=(j == 0),
                        stop=(j == TILES - 1),
                    )

            # write back partials (PSUM -> SBUF -> DRAM; DMA can't read PSUM)
            ev = accp.tile([P, NFREE], mybir.dt.float32, tag="ev")
            nc.vector.tensor_reduce(
                out=ev[:, D : D + 1],
                in_=s1cols[:],
                axis=mybir.AxisListType.X,
                op=mybir.AluOpType.add,
            )
            nc.vector.tensor_copy(out=ev[:, 0:D], in_=psum[:])
            nc.sync.dma_start(out=out_sums[:, 0:NFREE], in_=ev[:])
            nc.sync.dma_start(out=out_sums[:, NFREE:NOUT], in_=stats[:])

    nc.compile()
    return nc


def _get_program():
    if "nc" not in _CACHE:
        _CACHE["nc"] = _build_program()
    return _CACHE["nc"]


def _shard_by_label(labels_i: np.ndarray):
    """Class-aligned cuts balancing row counts.

    Returns (order, shard row-slices, base class per shard) or None if the
    label distribution cannot be packed into the compiled shard size.
    """
    counts = np.bincount(labels_i, minlength=C)
    cum = np.concatenate([[0], np.cumsum(counts)])  # [C+1]
    ntot = labels_i.shape[0]
    # cut k at the class boundary closest to k*ntot/8
    cuts = [0]
    for k in range(1, N_CORES):
        target = k * ntot / N_CORES
        c = int(np.searchsorted(cum, target))
        # nearest boundary
        if c > 0 and abs(cum[c - 1] - target) < abs(cum[c] - target):
            c -= 1
        c = min(max(c, cuts[-1]), C)
        cuts.append(c)
    cuts.append(C)
    spans = np.diff(cuts)
    rows = np.diff(cum[cuts])
    if spans.max() > P or rows.max() > BS_PAD:
        return None
    order = np.argsort(labels_i, kind="stable")
    row_slices = [(int(cum[cuts[k]]), int(cum[cuts[k + 1]])) for k in range(N_CORES)]
    return order, row_slices, cuts[:-1], spans


def _host_reference(feats, centers, labels_i):
    """Pure-host fallback for pathological label distributions that don't fit
    the compiled shard size (never triggered by uniform labels)."""
    f64 = feats.astype(np.float64)
    sums = np.zeros((C, D))
    np.add.at(sums, labels_i, f64)
    counts = np.bincount(labels_i, minlength=C).astype(np.float64)
    means = sums / np.maximum(counts, 1.0)[:, None]
    newc = np.where(
        (counts > 0)[:, None], 0.5 * centers.astype(np.float64) + 0.5 * means,
        centers.astype(np.float64),
    )
    return np.float32(0.5 * np.mean(((f64 - newc[labels_i]) ** 2).sum(1)))


def _run_device(in_maps, trace: bool = False):
    from concourse.bass_utils import run_bass_kernel_spmd

    nc = _get_program()
    kw = {"trace": True} if trace else {}
    try:
        return run_bass_kernel_spmd(nc, in_maps, core_ids=list(range(N_CORES)), **kw)
    except Exception:
        # transient axon/terminal faults have been observed; retry once
        import time

        time.sleep(2.0)
        return run_bass_kernel_spmd(nc, in_maps, core_ids=list(range(N_CORES)), **kw)


def kernel(feats, centers, labels, _trace: bool = False, _return_res: bool = False):
    feats = np.asarray(feats, dtype=np.float32)
    centers = np.asarray(centers, dtype=np.float32)
    labels_i = np.asarray(labels).astype(np.int64)

    sharding = _shard_by_label(labels_i)
    if sharding is None:
        return _host_reference(feats, centers, labels_i)
    order, row_slices, bases, spans = sharding

    in_maps = []
    for k in range(N_CORES):
        lo, hi = row_slices[k]
        idx = order[lo:hi]
        n = hi - lo
        f16 = np.zeros((BS_PAD, D), dtype=np.float16)
        f16[:n] = feats[idx]
        # pre-tile: [TILES, P, D] -> [P, TILES*D]
        ftile = np.ascontiguousarray(
            f16.reshape(TILES, P, D).transpose(1, 0, 2)
        ).reshape(P, TILES * D)
        rel = np.full(BS_PAD, PAD_LABEL, dtype=np.float16)
        rel[:n] = (labels_i[idx] - bases[k]).astype(np.float16)
        ltile = np.ascontiguousarray(rel.reshape(TILES, P).T)
        in_maps.append({"feats": ftile, "labels": ltile})

    res = _run_device(in_maps, trace=_trace)

    # host combine: concatenate per-core local sums (disjoint classes),
    # then the tiny [C, D] closed form in float64
    sums = np.zeros((C, D), dtype=np.float64)
    S1 = 0.0
    for k in range(N_CORES):
        raw = res.results[k]["out_sums"]
        span = int(spans[k])
        sums[bases[k] : bases[k] + span] = raw[:span, :D].astype(np.float64)
        S1 += float(raw[:, D].sum())
        # bn_stats words: [count, mean, count*var] x (even, odd) per sub-tile
        st = raw[:, NFREE:NOUT].astype(np.float64).reshape(P, -1, 3)
        cnt, mean, cvar = st[..., 0], st[..., 1], st[..., 2]
        S1 += float((cvar + cnt * mean * mean).sum())

    counts = np.bincount(labels_i, minlength=C).astype(np.float64)
    c64 = centers.astype(np.float64)
    A = float((sums * c64).sum())
    present = counts > 0
    X = float((np.square(sums).sum(axis=1)[present] / counts[present]).sum())
    W = float((counts * np.square(c64).sum(axis=1)).sum())
    loss = 0.5 / B * (S1 - 0.5 * A - 0.75 * X + 0.25 * W)
    out = np.float32(loss)
    if _return_res:
        return out, res
    return out


# revision 23
# speedup vs baseline: 3.6932x; 1.3620x over previous
"""CenterLoss kernel for Trainium2 (8 NeuronCores, label-range sharding).

Algorithm
---------
reference computes:
    counts[c] = #{i: y_i = c};  sums[c] = sum_{i: y_i = c} f_i
    means = sums / max(counts, 1);  present = counts > 0
    n_c = present ? 0.5*centers_c + 0.5*means_c : centers_c
    loss = 0.5 * mean_i ||f_i - n_{y_i}||^2

Expanding the loss (every class that appears in the batch is present):
    B * 2 * loss = S1 - 0.5*A - 0.75*X + 0.25*W
where
    S1 = sum_i ||f_i||^2
    A  = sum_c sums_c . centers_c
    X  = sum_{c present} ||sums_c||^2 / counts_c
    W  = sum_c counts_c * ||centers_c||^2

Device work: segment sums over feats (B=131072, D=256, C=1000) and S1.

Sharding: rows are sharded by LABEL RANGE (class-aligned cuts balancing row
counts, ~125 classes / ~16384 rows per core).  Every row in a core then has a
label inside one 128-wide class window, so the one-hot segment-sum matmul
needs a single [128rows x 128cls] stationary per 128-row tile accumulating
into ONE [128, 256] PSUM tile -- 8x less PE work than padding the one-hot to
1024 classes, and no inter-core reduction (classes are disjoint; the host
just concatenates the per-core sums).  counts come from a host bincount.

feats are staged to HBM as fp16 (exact enough: final rel err ~1e-6 vs the
fp32 reference; the check budget is 2e-2), which halves the HBM traffic; the
kernel is then DMA-bound at ~8.7 MB / 360 GB/s ~= 24 us per core.  S1
squares are split ACT/DVE and one-hot builds split DVE/Pool so every
compute engine stays under the DMA window.
"""

import sys

sys.path.insert(0, "/opt/trn_rl_repo")

import numpy as np

# problem shape (hardcoded per the harness contract)
B, D, C = 131072, 256, 1000
N_CORES = 8
P = 128
TG = 4  # row-tiles per DMA group
BS_PAD = 16896  # padded rows per core (16384 + 512 slack for shard imbalance)
TILES = BS_PAD // P  # 132
GROUPS = TILES // TG  # 33
NFREE = D + 1  # 256 sums cols + 1 S1 col in the output (legacy name)
PAD_LABEL = 127.0  # relative label for padded rows (feats are 0 -> no-op)

# engine split knobs (tuned against the TimelineSim trace)
# 14 DVE-square groups spread over the first 31; the last 2 go to ACT so the
# deferred bn_stats tail never serializes after the final DMA
SQ_ON_DVE = frozenset(
    g
    for g in range(GROUPS - 2)
    if g * 14 // (GROUPS - 2) != (g + 1) * 14 // (GROUPS - 2)
)
OH_POOL_PER_GROUP = 2  # one-hots per group built on Pool (rest on DVE)
SQ_LAG = 2  # DVE bn_stats issued this many groups late so they never block
N_DVE = len(SQ_ON_DVE)
N_ACT = GROUPS - N_DVE
NSTAT = 12 * N_DVE  # bn_stats words shipped per partition (2 x [P,512] calls)
NOUT = D + N_ACT + NSTAT

_CACHE: dict = {}


def _build_program():
    import concourse.bacc as bacc
    import concourse.bass as bass
    from concourse import mybir
    from concourse.tile import TileContext

    nc = bacc.Bacc("TRN2", target_bir_lowering=False)

    # feats pre-tiled on host: [P, TILES*D], row p holds tile-rows
    # (t*128+p for all t) concatenated -> group loads are 1 descriptor
    # per partition (TG*D*2 = 2 KB contiguous)
    feats = nc.dram_tensor(
        "feats", [P, TILES * D], mybir.dt.float16, kind="ExternalInput"
    )
    labels_in = nc.dram_tensor(
        "labels", [P, TILES], mybir.dt.float16, kind="ExternalInput"
    )
    # [128 x (256 local-class sums | N_ACT s1 columns | bn_stats words)]
    out_sums = nc.dram_tensor(
        "out_sums", [P, NOUT], mybir.dt.float32, kind="ExternalOutput"
    )

    feats_ap = feats[:]

    with TileContext(nc) as tc:
        with (
            tc.tile_pool(name="const", bufs=1) as const,
            tc.tile_pool(name="fin", bufs=8) as fin,
            tc.tile_pool(name="sq", bufs=2) as sqp,
            tc.tile_pool(name="ohp", bufs=12) as ohp,
            tc.tile_pool(name="accp", bufs=1) as accp,
            tc.tile_pool(name="psp", bufs=1, space="PSUM") as psp,
        ):
            # labels tiles (the DMA is issued inside the group loop right
            # after feats group 0 so the feats stream owns the head of the
            # DMA queue); converted to fp32 on DVE because tensor_scalar
            # is_equal needs an fp32 scalar operand
            labels16_t = const.tile([P, TILES], mybir.dt.float16, tag="labels16_t")
            labels_t = const.tile([P, TILES], mybir.dt.float32, tag="labels_t")
            iota_i = const.tile([P, P], mybir.dt.int32, tag="iota_i")
            nc.gpsimd.iota(iota_i[:], pattern=[[1, P]], channel_multiplier=0)
            iota_f = const.tile([P, P], mybir.dt.float16, tag="iota_f")
            nc.vector.tensor_copy(out=iota_f[:], in_=iota_i[:])
            iota_t = iota_f[:]

            # persistent accumulators (s1cols: one column per ACT group;
            # DVE groups ship raw bn_stats instead)
            s1cols = accp.tile([P, N_ACT], mybir.dt.float32, tag="s1cols")
            stats = accp.tile([P, 2 * N_DVE, 6], mybir.dt.float32, tag="stats")
            # bn_stats record layout can vary with AP lowering; zero-fill so
            # unwritten slots contribute 0 to the host-side sum(x^2)
            nc.vector.memset(stats[:], 0.0)
            psum = psp.tile([P, D], mybir.dt.float32, tag="psum", name="psum")

            # HAM warm-up: the PE p-state ramps with sustained activity; issue
            # dummy matmuls early so the real stream runs at full clock.
            # Results land in psum but are discarded by the first start=True.
            warm = const.tile([P, D], mybir.dt.float16, tag="warm")
            nc.vector.memset(warm[:], 0.0)
            for w in range(5):
                nc.tensor.matmul(
                    out=psum[:],
                    lhsT=warm[:, 0:P],
                    rhs=warm[:],
                    start=True,
                    stop=True,
                )

            act_col = 0
            dve_idx = 0
            fgs = {}

            def emit_dve_square(g):
                nonlocal dve_idx
                for h in range(2):
                    nc.vector.bn_stats(
                        out=stats[:, dve_idx * 2 + h],
                        in_=fgs[g][:, h * 2 * D : (h + 1) * 2 * D],
                    )
                dve_idx += 1

            for g in range(GROUPS):
                # load a [P, TG*D] group of tile-rows (1 descriptor/partition)
                fg = fin.tile([P, TG * D], mybir.dt.float16, tag="fg", name="fg")
                fgs[g] = fg
                nc.sync.dma_start(
                    out=fg[:],
                    in_=bass.AP(
                        tensor=feats_ap.tensor,
                        offset=g * TG * D,
                        ap=[[TILES * D, P], [1, TG * D]],
                    ),
                )
                if g == 0:
                    nc.sync.dma_start(out=labels16_t[:], in_=labels_in[:])
                    nc.vector.tensor_copy(out=labels_t[:], in_=labels16_t[:])
                # S1 partial: ACT groups do Square+accum inline; DVE groups
                # emit bn_stats ([count, mean, count*var] per [P,512] half,
                # sum(x^2) recovered on host) SQ_LAG groups late so the DMA
                # wait never stalls younger one-hots in the in-order DVE queue
                if g not in SQ_ON_DVE:
                    sqt = sqp.tile([P, TG * D], mybir.dt.float16, tag="sqt", name="sqt")
                    nc.scalar.activation(
                        out=sqt[:],
                        in_=fg[:],
                        func=mybir.ActivationFunctionType.Square,
                        accum_out=s1cols[:, act_col : act_col + 1],
                    )
                    act_col += 1
                # one-hots (DVE at 4x; a share on Pool) + segment matmuls
                for s in range(TG):
                    j = g * TG + s
                    oh = ohp.tile([P, P], mybir.dt.float16, tag="oh")
                    eng = nc.gpsimd if s < OH_POOL_PER_GROUP else nc.vector
                    eng.tensor_scalar(
                        oh[:],
                        iota_t,
                        labels_t[:, j : j + 1],
                        None,
                        mybir.AluOpType.is_equal,
                    )
                    nc.tensor.matmul(
                        out=psum[:],
                        lhsT=oh[:],
                        rhs=fg[:, s * D : (s + 1) * D],
                        start=(j == 0),
                        stop=(j == TILES - 1),
                    )
                if g >= SQ_LAG and (g - SQ_LAG) in SQ_ON_DVE:
                    emit_dve_square(g - SQ_LAG)
            for g in range(GROUPS - SQ_LAG, GROUPS):
                if g in SQ_ON_DVE:
                    emit_dve_square(g)

            # write back partials; one store per producer on three separate
            # DGE rings so their 565ns sequencer configs don't serialize
            # (PSUM -> SBUF -> DRAM; DMA can't read PSUM)
            nc.sync.dma_start(out=out_sums[:, D + N_ACT : NOUT], in_=stats[:])
            nc.sync.dma_start(out=out_sums[:, D : D + N_ACT], in_=s1cols[:])
            ev = accp.tile([P, D], mybir.dt.float32, tag="ev")
            nc.vector.tensor_copy(out=ev[:], in_=psum[:])
            nc.sync.dma_start(out=out_sums[:, 0:D], in_=ev[:])

    nc.compile()
    return nc


def _get_program():
    if "nc" not in _CACHE:
        _CACHE["nc"] = _build_program()
    return _CACHE["nc"]


def _shard_by_label(labels_i: np.ndarray):
    """Class-aligned cuts balancing row counts.

    Returns (order, shard row-slices, base class per shard) or None if the
    label distribution cannot be packed into the compiled shard size.
    """
    counts = np.bincount(labels_i, minlength=C)
    cum = np.concatenate([[0], np.cumsum(counts)])  # [C+1]
    ntot = labels_i.shape[0]
    # cut k at the class boundary closest to k*ntot/8
    cuts = [0]
    for k in range(1, N_CORES):
        target = k * ntot / N_CORES
        c = int(np.searchsorted(cum, target))
        # nearest boundary
        if c > 0 and abs(cum[c - 1] - target) < abs(cum[c] - target):
            c -= 1
        c = min(max(c, cuts[-1]), C)
        cuts.append(c)
    cuts.append(C)
    spans = np.diff(cuts)
    rows = np.diff(cum[cuts])
    if spans.max() > P or rows.max() > BS_PAD:
        return None
    order = np.argsort(labels_i, kind="stable")
    row_slices = [(int(cum[cuts[k]]), int(cum[cuts[k + 1]])) for k in range(N_CORES)]
    return order, row_slices, cuts[:-1], spans


def _host_reference(feats, centers, labels_i):
    """Pure-host fallback for pathological label distributions that don't fit
    the compiled shard size (never triggered by uniform labels)."""
    f64 = feats.astype(np.float64)
    sums = np.zeros((C, D))
    np.add.at(sums, labels_i, f64)
    counts = np.bincount(labels_i, minlength=C).astype(np.float64)
    means = sums / np.maximum(counts, 1.0)[:, None]
    newc = np.where(
        (counts > 0)[:, None], 0.5 * centers.astype(np.float64) + 0.5 * means,
        centers.astype(np.float64),
    )
    return np.float32(0.5 * np.mean(((f64 - newc[labels_i]) ** 2).sum(1)))


def _run_device(in_maps, trace: bool = False):
    from concourse.bass_utils import run_bass_kernel_spmd

    nc = _get_program()
    kw = {"trace": True} if trace else {}
    try:
        return run_bass_kernel_spmd(nc, in_maps, core_ids=list(range(N_CORES)), **kw)
    except Exception:
        # transient axon/terminal faults have been observed; retry once
        import time

        time.sleep(2.0)
        return run_bass_kernel_spmd(nc, in_maps, core_ids=list(range(N_CORES)), **kw)


def kernel(feats, centers, labels, _trace: bool = False, _return_res: bool = False):
    feats = np.asarray(feats, dtype=np.float32)
    centers = np.asarray(centers, dtype=np.float32)
    labels_i = np.asarray(labels).astype(np.int64)

    sharding = _shard_by_label(labels_i)
    if sharding is None:
        return _host_reference(feats, centers, labels_i)
    order, row_slices, bases, spans = sharding

    in_maps = []
    for k in range(N_CORES):
        lo, hi = row_slices[k]
        idx = order[lo:hi]
        n = hi - lo
        f16 = np.zeros((BS_PAD, D), dtype=np.float16)
        f16[:n] = feats[idx]
        # pre-tile: [TILES, P, D] -> [P, TILES*D]
        ftile = np.ascontiguousarray(
            f16.reshape(TILES, P, D).transpose(1, 0, 2)
        ).reshape(P, TILES * D)
        rel = np.full(BS_PAD, PAD_LABEL, dtype=np.float16)
        rel[:n] = (labels_i[idx] - bases[k]).astype(np.float16)
        ltile = np.ascontiguousarray(rel.reshape(TILES, P).T)
        in_maps.append({"feats": ftile, "labels": ltile})

    res = _run_device(in_maps, trace=_trace)

    # host combine: concatenate per-core local sums (disjoint classes),
    # then the tiny [C, D] closed form in float64
    sums = np.zeros((C, D), dtype=np.float64)
    S1 = 0.0
    for k in range(N_CORES):
        raw = res.results[k]["out_sums"]
        span = int(spans[k])
        sums[bases[k] : bases[k] + span] = raw[:span, :D].astype(np.float64)
        S1 += float(raw[:, D : D + N_ACT].sum())
        # bn_stats words: [count, mean, count*var] x (even, odd) halves
        st = raw[:, D + N_ACT : NOUT].astype(np.float64).reshape(P, -1, 3)
        cnt, mean, cvar = st[..., 0], st[..., 1], st[..., 2]
        S1 += float((cvar + cnt * mean * mean).sum())

    counts = np.bincount(labels_i, minlength=C).astype(np.float64)
    c64 = centers.astype(np.float64)
    A = float((sums * c64).sum())
    present = counts > 0
    X = float((np.square(sums).sum(axis=1)[present] / counts[present]).sum())
    W = float((counts * np.square(c64).sum(axis=1)).sum())
    loss = 0.5 / B * (S1 - 0.5 * A - 0.75 * X + 0.25 * W)
    out = np.float32(loss)
    if _return_res:
        return out, res
    return out
